# revision 1
# baseline (speedup 1.0000x reference)
"""Top-1 (Switch) MoE layer on 8 Trainium2 NeuronCores — expert parallelism.

Sharding strategy:
  - Each core e owns expert e's weights (wi[e], wo[e]) — expert parallel.
  - Host computes the dispatch (router argmax) and all-to-alls the tokens:
    core e receives the tokens routed to expert e (padded to a fixed
    capacity), transposed to [d_model, capacity] so the device needs no
    transposes.
  - Router weights are replicated; each core re-computes router logits
    on-device for (a) its contiguous 256-token shard (for the full
    router_logits / expert_index outputs) in plain fp32 so argmax matches
    the reference bit-for-bit, and (b) its gathered tokens (for the top-1
    probability that scales the FFN output).
  - FFN matmuls run in float32r (full-rate fp32 mode on the PE).
  - Host scatters each core's [capacity, d_model] result back to token
    order (pure data movement) and concatenates the shard outputs.
"""

import numpy as np

NUM_EXPERTS = 8
D_MODEL = 512
D_FF = 2048
BATCH, SEQ = 1, 2048
N_CORES = 8
SHARD = SEQ // N_CORES          # 256 router tokens per core
KC = D_MODEL // 128             # 4 contraction chunks of d_model
NF = D_FF // 128                # 16 chunks of d_ff
NS = SHARD // 128               # 2 shard tiles

_CACHE = {}
LAST_RESULTS = None
PROFILE = False
TRACE_CORES = None


def _build(c_pad):
    import concourse.bacc as bacc
    import concourse.bass as bass
    import concourse.tile as tile
    from concourse import mybir

    f32 = mybir.dt.float32
    f32r = mybir.dt.float32r
    i32 = mybir.dt.int32
    AF = mybir.ActivationFunctionType
    ALU = mybir.AluOpType
    AX = mybir.AxisListType
    ts = bass.ts

    nm = c_pad // 128
    nc = bacc.Bacc(None, target_bir_lowering=False)

    xg_t = nc.dram_tensor("xg_t", [D_MODEL, c_pad], f32r, kind="ExternalInput")
    xs_t = nc.dram_tensor("xs_t", [D_MODEL, SHARD], f32, kind="ExternalInput")
    wr = nc.dram_tensor("wr", [D_MODEL, NUM_EXPERTS], f32, kind="ExternalInput")
    wr_r = nc.dram_tensor("wr_r", [D_MODEL, NUM_EXPERTS], f32r, kind="ExternalInput")
    wi = nc.dram_tensor("wi", [D_MODEL, D_FF], f32r, kind="ExternalInput")
    wo = nc.dram_tensor("wo", [D_FF, D_MODEL], f32r, kind="ExternalInput")

    y_out = nc.dram_tensor("y_out", [nm, 128, D_MODEL], f32, kind="ExternalOutput")
    lg_out = nc.dram_tensor("lg_out", [NS, 128, NUM_EXPERTS], f32, kind="ExternalOutput")
    ix_out = nc.dram_tensor("ix_out", [NS, 128], i32, kind="ExternalOutput")

    with tile.TileContext(nc) as tc:
        with (
            tc.tile_pool(name="weights", bufs=1) as wpool,
            tc.tile_pool(name="acts", bufs=1) as apool,
            tc.tile_pool(name="small", bufs=2) as spool,
            tc.tile_pool(name="psum_r", bufs=2, space="PSUM") as prpool,
            tc.tile_pool(name="psum_h", bufs=2, space="PSUM") as phpool,
            tc.tile_pool(name="psum_y", bufs=2, space="PSUM") as pypool,
        ):
            # descending iota 7..0, used to pick the FIRST max (argmax)
            iota_i = wpool.tile([128, NUM_EXPERTS], i32, tag="iota_i")
            nc.gpsimd.iota(
                iota_i[:, :], pattern=[[-1, NUM_EXPERTS]], base=7, channel_multiplier=0
            )
            iota_f = wpool.tile([128, NUM_EXPERTS], f32, tag="iota_f")
            nc.vector.tensor_copy(iota_f[:, :], iota_i[:, :])

            wr_sb = wpool.tile([128, KC, NUM_EXPERTS], f32, tag="wr")
            nc.sync.dma_start(out=wr_sb[:], in_=wr.rearrange("(c p) e -> p c e", p=128))
            wrr_sb = wpool.tile([128, KC, NUM_EXPERTS], f32r, tag="wrr")
            nc.sync.dma_start(
                out=wrr_sb[:], in_=wr_r.rearrange("(c p) e -> p c e", p=128)
            )
            xs_sb = apool.tile([128, KC, SHARD], f32, tag="xs")
            nc.sync.dma_start(out=xs_sb[:], in_=xs_t.rearrange("(c p) n -> p c n", p=128))
            xg_sb = apool.tile([128, KC, c_pad], f32r, tag="xg")
            nc.sync.dma_start(out=xg_sb[:], in_=xg_t.rearrange("(c p) n -> p c n", p=128))

            wi_sb = wpool.tile([128, KC, D_FF], f32r, tag="wi")
            wi_r = wi.rearrange("(c p) f -> p c f", p=128)
            for c in range(KC):
                nc.sync.dma_start(out=wi_sb[:, c, :], in_=wi_r[:, c, :])
            wo_sb = wpool.tile([128, NF, D_MODEL], f32r, tag="wo")
            wo_r = wo.rearrange("(g p) d -> p g d", p=128)
            for g in range(4):
                nc.sync.dma_start(
                    out=wo_sb[:, 4 * g : 4 * (g + 1), :], in_=wo_r[:, 4 * g : 4 * (g + 1), :]
                )

            # ---- router on the contiguous shard (fp32, exact argmax) ----
            lg_sb = apool.tile([128, NS, NUM_EXPERTS], f32, tag="lg")
            ix_sb = apool.tile([128, NS], i32, tag="ix")
            for m in range(NS):
                pl = prpool.tile([128, NUM_EXPERTS], f32, tag="pr")
                for c in range(KC):
                    nc.tensor.matmul(
                        pl[:, :],
                        xs_sb[:, c, ts(m, 128)],
                        wr_sb[:, c, :],
                        start=(c == 0),
                        stop=(c == KC - 1),
                    )
                nc.vector.tensor_copy(lg_sb[:, m, :], pl[:, :])
                lmax = spool.tile([128, 1], f32, tag="lmax")
                nc.vector.tensor_reduce(lmax[:, :], pl[:, :], axis=AX.X, op=ALU.max)
                eq = spool.tile([128, NUM_EXPERTS], f32, tag="eq")
                nc.vector.tensor_scalar(eq[:, :], pl[:, :], lmax[:, :], None, ALU.is_equal)
                nc.vector.tensor_mul(eq[:, :], eq[:, :], iota_f[:, :])
                m7 = spool.tile([128, 1], f32, tag="m7")
                nc.vector.tensor_reduce(m7[:, :], eq[:, :], axis=AX.X, op=ALU.max)
                idxf = spool.tile([128, 1], f32, tag="idxf")
                nc.vector.tensor_scalar(
                    idxf[:, :], m7[:, :], -1.0, 7.0, ALU.mult, ALU.add
                )
                nc.vector.tensor_copy(ix_sb[:, m : m + 1], idxf[:, :])
            nc.sync.dma_start(out=lg_out.rearrange("m p e -> p m e"), in_=lg_sb[:])
            nc.sync.dma_start(out=ix_out.rearrange("m p -> p m"), in_=ix_sb[:])

            # ---- top-1 probability for the gathered tokens ----
            prob_sb = apool.tile([128, nm], f32, tag="prob")
            for m in range(nm):
                pg = prpool.tile([128, NUM_EXPERTS], f32, tag="pr")
                for c in range(KC):
                    nc.tensor.matmul(
                        pg[:, :],
                        xg_sb[:, c, ts(m, 128)],
                        wrr_sb[:, c, :],
                        start=(c == 0),
                        stop=(c == KC - 1),
                    )
                gmax = spool.tile([128, 1], f32, tag="gmax")
                nc.vector.tensor_reduce(gmax[:, :], pg[:, :], axis=AX.X, op=ALU.max)
                ls = spool.tile([128, NUM_EXPERTS], f32, tag="ls")
                nc.vector.tensor_scalar(ls[:, :], pg[:, :], gmax[:, :], None, ALU.subtract)
                ex = spool.tile([128, NUM_EXPERTS], f32, tag="ex")
                esum = spool.tile([128, 1], f32, tag="esum")
                nc.scalar.activation(ex[:, :], ls[:, :], AF.Exp, accum_out=esum[:, :])
                nc.vector.reciprocal(prob_sb[:, m : m + 1], esum[:, :])

            # ---- FFN1: hT[f-chunk, tokens] = relu(wi_chunk.T @ xg) ----
            h_sb = apool.tile([128, NF, c_pad], f32r, tag="h")
            for f in range(NF):
                ph = phpool.tile([128, c_pad], f32, tag="ph")
                for c in range(KC):
                    nc.tensor.matmul(
                        ph[:, :],
                        wi_sb[:, c, ts(f, 128)],
                        xg_sb[:, c, :],
                        start=(c == 0),
                        stop=(c == KC - 1),
                    )
                nc.scalar.activation(h_sb[:, f, :], ph[:, :], AF.Relu)

            # ---- FFN2: y[tokens, d] = prob * (h @ wo) ----
            y_sb = apool.tile([128, nm, D_MODEL], f32, tag="y")
            for m in range(nm):
                py = pypool.tile([128, D_MODEL], f32, tag="py")
                for f in range(NF):
                    nc.tensor.matmul(
                        py[:, :],
                        h_sb[:, f, ts(m, 128)],
                        wo_sb[:, f, :],
                        start=(f == 0),
                        stop=(f == NF - 1),
                    )
                nc.vector.tensor_scalar(
                    y_sb[:, m, :], py[:, :], prob_sb[:, m : m + 1], None, ALU.mult
                )
            nc.sync.dma_start(out=y_out.rearrange("m p d -> p m d"), in_=y_sb[:])

    nc.compile()
    return nc


def _dispatch(x, wr):
    """Host-side all-to-all dispatch decision: token -> expert."""
    logits = x @ wr
    eidx = np.argmax(logits, axis=-1)
    counts = np.bincount(eidx, minlength=NUM_EXPERTS)
    c_pad = max(256, -(-int(counts.max()) // 128) * 128)
    return eidx, counts, c_pad


def kernel(hidden_states, w_router, wi, wo):
    x = np.ascontiguousarray(
        np.asarray(hidden_states, dtype=np.float32).reshape(SEQ, D_MODEL)
    )
    wr = np.ascontiguousarray(np.asarray(w_router, dtype=np.float32))
    wi = np.asarray(wi, dtype=np.float32)
    wo = np.asarray(wo, dtype=np.float32)

    eidx, counts, c_pad = _dispatch(x, wr)

    nc = _CACHE.get(c_pad)
    if nc is None:
        nc = _build(c_pad)
        _CACHE[c_pad] = nc

    tok_lists = [np.nonzero(eidx == e)[0] for e in range(NUM_EXPERTS)]
    in_maps = []
    for e in range(NUM_EXPERTS):
        toks = tok_lists[e]
        xg = np.zeros((c_pad, D_MODEL), np.float32)
        xg[: len(toks)] = x[toks]
        in_maps.append(
            {
                "xg_t": np.ascontiguousarray(xg.T),
                "xs_t": np.ascontiguousarray(x[e * SHARD : (e + 1) * SHARD].T),
                "wr": wr,
                "wr_r": wr,
                "wi": np.ascontiguousarray(wi[e]),
                "wo": np.ascontiguousarray(wo[e]),
            }
        )

    from concourse.bass_utils import run_bass_kernel_spmd

    global LAST_RESULTS
    res = run_bass_kernel_spmd(
        nc,
        in_maps,
        list(range(N_CORES)),
        trace=PROFILE,
        trace_cores=TRACE_CORES,
    )
    LAST_RESULTS = res

    out = np.empty((SEQ, D_MODEL), np.float32)
    lg_full = np.empty((SEQ, NUM_EXPERTS), np.float32)
    ix_full = np.empty((SEQ,), np.int32)
    for e in range(NUM_EXPERTS):
        r = res.results[e]
        toks = tok_lists[e]
        out[toks] = r["y_out"].reshape(-1, D_MODEL)[: len(toks)]
        lg_full[e * SHARD : (e + 1) * SHARD] = r["lg_out"].reshape(SHARD, NUM_EXPERTS)
        ix_full[e * SHARD : (e + 1) * SHARD] = r["ix_out"].reshape(SHARD)

    return (
        out.reshape(BATCH, SEQ, D_MODEL),
        (
            lg_full.reshape(BATCH, SEQ, NUM_EXPERTS),
            ix_full.reshape(BATCH, SEQ),
        ),
    )


# revision 5
# speedup vs baseline: 1.1001x; 1.1001x over previous
"""Top-1 (Switch) MoE layer on 8 Trainium2 NeuronCores — expert parallelism.

Sharding strategy:
  - Each core e owns expert e's weights (wi[e], wo[e]) — expert parallel.
  - Host computes the dispatch (router argmax) and all-to-alls the tokens:
    core e receives the tokens routed to expert e (padded to a fixed
    capacity), transposed to [d_model, capacity] so the device needs no
    transposes.
  - Router weights are replicated; each core re-computes router logits
    on-device for (a) its contiguous 256-token shard (for the full
    router_logits / expert_index outputs) in plain fp32 so argmax matches
    the reference bit-for-bit, and (b) its gathered tokens (for the top-1
    probability that scales the FFN output).
  - FFN matmuls run in fp16 by default (fast weight load + half the HBM
    traffic); the router stays fp32.
  - FFN1 and FFN2 are interleaved at d_ff-chunk granularity so wo can
    stream in behind wi and the PE stays dense (HAM ramps early).
  - Host scatters each core's [capacity, d_model] result back to token
    order (pure data movement) and concatenates the shard outputs.
"""

import numpy as np

NUM_EXPERTS = 8
D_MODEL = 512
D_FF = 2048
BATCH, SEQ = 1, 2048
N_CORES = 8
SHARD = SEQ // N_CORES          # 256 router tokens per core
KC = D_MODEL // 128             # 4 contraction chunks of d_model
NF = D_FF // 128                # 16 chunks of d_ff
NG = 4                          # d_ff chunk groups (stream granularity)
NFG = NF // NG                  # f-chunks per group
NS = SHARD // 128               # 2 shard tiles

DTYPE = "fp16"                  # "fp16" | "bf16" | "f32r"

_CACHE = {}
LAST_RESULTS = None
PROFILE = False
TRACE_CORES = None


def _build(c_pad, dtype_name):
    import concourse.bacc as bacc
    import concourse.bass as bass
    import concourse.tile as tile
    from concourse import mybir

    f32 = mybir.dt.float32
    i32 = mybir.dt.int32
    ff = {
        "fp16": mybir.dt.float16,
        "bf16": mybir.dt.bfloat16,
        "f32r": mybir.dt.float32r,
    }[dtype_name]
    AF = mybir.ActivationFunctionType
    ALU = mybir.AluOpType
    AX = mybir.AxisListType
    ts = bass.ts

    nm = c_pad // 128
    nc = bacc.Bacc(None, target_bir_lowering=False)

    xg_t = nc.dram_tensor("xg_t", [D_MODEL, c_pad], ff, kind="ExternalInput")
    xs_t = nc.dram_tensor("xs_t", [D_MODEL, SHARD], f32, kind="ExternalInput")
    wr = nc.dram_tensor("wr", [D_MODEL, NUM_EXPERTS], f32, kind="ExternalInput")
    wr_r = nc.dram_tensor("wr_r", [D_MODEL, NUM_EXPERTS], ff, kind="ExternalInput")
    wi = nc.dram_tensor("wi", [D_MODEL, D_FF], ff, kind="ExternalInput")
    wo = nc.dram_tensor("wo", [D_FF, D_MODEL], ff, kind="ExternalInput")

    y_out = nc.dram_tensor("y_out", [nm, 128, D_MODEL], f32, kind="ExternalOutput")
    lg_out = nc.dram_tensor("lg_out", [NS, 128, NUM_EXPERTS], f32, kind="ExternalOutput")
    ix_out = nc.dram_tensor("ix_out", [NS, 128], i32, kind="ExternalOutput")

    with tile.TileContext(nc) as tc:
        with (
            tc.tile_pool(name="weights", bufs=1) as wpool,
            tc.tile_pool(name="acts", bufs=1) as apool,
            tc.tile_pool(name="hbuf", bufs=3) as hpool,
            tc.tile_pool(name="small", bufs=2) as spool,
            tc.tile_pool(name="psum_r", bufs=2, space="PSUM") as prpool,
            tc.tile_pool(name="psum_h", bufs=2, space="PSUM") as phpool,
            tc.tile_pool(name="psum_y", bufs=1, space="PSUM") as pypool,
        ):
            # ---- input DMAs: small/early ones on the scalar queue ----
            wr_sb = wpool.tile([128, KC, NUM_EXPERTS], f32, tag="wr")
            nc.scalar.dma_start(out=wr_sb[:], in_=wr.rearrange("(c p) e -> p c e", p=128))
            xs_sb = apool.tile([128, KC, SHARD], f32, tag="xs")
            nc.scalar.dma_start(out=xs_sb[:], in_=xs_t.rearrange("(c p) n -> p c n", p=128))
            wrr_sb = wpool.tile([128, KC, NUM_EXPERTS], ff, tag="wrr")
            nc.scalar.dma_start(out=wrr_sb[:], in_=wr_r.rearrange("(c p) e -> p c e", p=128))
            xg_sb = apool.tile([128, KC, c_pad], ff, tag="xg")
            nc.scalar.dma_start(out=xg_sb[:], in_=xg_t.rearrange("(c p) n -> p c n", p=128))

            # wi f-chunk groups stream on the sync queue, wo on vector's
            wi_r = wi.rearrange("(c p) f -> p c f", p=128)
            wi_g = []
            for g in range(NG):
                t = wpool.tile([128, KC, NFG * 128], ff, tag=f"wi{g}")
                nc.sync.dma_start(out=t[:], in_=wi_r[:, :, ts(g, NFG * 128)])
                wi_g.append(t)
            wo_r = wo.rearrange("(f p) d -> p f d", p=128)
            wo_g = []
            for g in range(NG):
                t = wpool.tile([128, NFG, D_MODEL], ff, tag=f"wo{g}")
                nc.scalar.dma_start(out=t[:], in_=wo_r[:, ts(g, NFG), :])
                wo_g.append(t)

            # descending iota 7..0, used to pick the FIRST max (argmax)
            iota_i = wpool.tile([128, NUM_EXPERTS], i32, tag="iota_i")
            nc.gpsimd.iota(
                iota_i[:, :], pattern=[[-1, NUM_EXPERTS]], base=7, channel_multiplier=0
            )
            iota_f = wpool.tile([128, NUM_EXPERTS], f32, tag="iota_f")
            nc.vector.tensor_copy(iota_f[:, :], iota_i[:, :])

            # ---- router on the contiguous shard (fp32, exact argmax) ----
            lg_sb = apool.tile([128, NS, NUM_EXPERTS], f32, tag="lg")
            ix_sb = apool.tile([128, NS], i32, tag="ix")
            for m in range(NS):
                pl = prpool.tile([128, NUM_EXPERTS], f32, tag="pr")
                for c in range(KC):
                    nc.tensor.matmul(
                        pl[:, :],
                        xs_sb[:, c, ts(m, 128)],
                        wr_sb[:, c, :],
                        start=(c == 0),
                        stop=(c == KC - 1),
                    )
                nc.vector.tensor_copy(lg_sb[:, m, :], pl[:, :])
                lmax = spool.tile([128, 1], f32, tag="lmax")
                nc.vector.tensor_reduce(lmax[:, :], pl[:, :], axis=AX.X, op=ALU.max)
                eq = spool.tile([128, NUM_EXPERTS], f32, tag="eq")
                nc.vector.tensor_scalar(eq[:, :], pl[:, :], lmax[:, :], None, ALU.is_equal)
                nc.vector.tensor_mul(eq[:, :], eq[:, :], iota_f[:, :])
                m7 = spool.tile([128, 1], f32, tag="m7")
                nc.vector.tensor_reduce(m7[:, :], eq[:, :], axis=AX.X, op=ALU.max)
                idxf = spool.tile([128, 1], f32, tag="idxf")
                nc.vector.tensor_scalar(
                    idxf[:, :], m7[:, :], -1.0, 7.0, ALU.mult, ALU.add
                )
                nc.vector.tensor_copy(ix_sb[:, m : m + 1], idxf[:, :])
            nc.gpsimd.dma_start(out=lg_out.rearrange("m p e -> p m e"), in_=lg_sb[:])
            nc.gpsimd.dma_start(out=ix_out.rearrange("m p -> p m"), in_=ix_sb[:])

            # ---- top-1 probability for the gathered tokens ----
            prob_sb = apool.tile([128, nm], f32, tag="prob")
            for m in range(nm):
                pg = prpool.tile([128, NUM_EXPERTS], f32, tag="pr")
                for c in range(KC):
                    nc.tensor.matmul(
                        pg[:, :],
                        xg_sb[:, c, ts(m, 128)],
                        wrr_sb[:, c, :],
                        start=(c == 0),
                        stop=(c == KC - 1),
                    )
                gmax = spool.tile([128, 1], f32, tag="gmax")
                nc.vector.tensor_reduce(gmax[:, :], pg[:, :], axis=AX.X, op=ALU.max)
                ls = spool.tile([128, NUM_EXPERTS], f32, tag="ls")
                nc.vector.tensor_scalar(ls[:, :], pg[:, :], gmax[:, :], None, ALU.subtract)
                ex = spool.tile([128, NUM_EXPERTS], f32, tag="ex")
                esum = spool.tile([128, 1], f32, tag="esum")
                nc.scalar.activation(ex[:, :], ls[:, :], AF.Exp, accum_out=esum[:, :])
                nc.vector.reciprocal(prob_sb[:, m : m + 1], esum[:, :])

            # ---- FFN1 + FFN2 interleaved per d_ff chunk ----
            py_tiles = [
                pypool.tile([128, D_MODEL], f32, tag=f"py{m}", name=f"py{m}")
                for m in range(nm)
            ]
            for g in range(NG):
                for fl in range(NFG):
                    f = g * NFG + fl
                    ph = phpool.tile([128, c_pad], f32, tag="ph")
                    for c in range(KC):
                        nc.tensor.matmul(
                            ph[:, :],
                            wi_g[g][:, c, ts(fl, 128)],
                            xg_sb[:, c, :],
                            start=(c == 0),
                            stop=(c == KC - 1),
                        )
                    hf = hpool.tile([128, c_pad], ff, tag="hf")
                    nc.scalar.activation(hf[:, :], ph[:, :], AF.Relu)
                    for m in range(nm):
                        nc.tensor.matmul(
                            py_tiles[m][:, :],
                            hf[:, ts(m, 128)],
                            wo_g[g][:, fl, :],
                            start=(f == 0),
                            stop=(f == NF - 1),
                        )

            y_sb = apool.tile([128, nm, D_MODEL], f32, tag="y")
            for m in range(nm):
                nc.vector.tensor_scalar(
                    y_sb[:, m, :], py_tiles[m][:, :], prob_sb[:, m : m + 1], None, ALU.mult
                )
                nc.gpsimd.dma_start(
                    out=y_out[m].rearrange("p d -> p d"), in_=y_sb[:, m, :]
                )

    nc.compile()
    return nc


def _dispatch(x, wr):
    """Host-side all-to-all dispatch decision: token -> expert."""
    logits = x @ wr
    eidx = np.argmax(logits, axis=-1)
    counts = np.bincount(eidx, minlength=NUM_EXPERTS)
    c_pad = max(256, -(-int(counts.max()) // 128) * 128)
    return eidx, counts, c_pad


def kernel(hidden_states, w_router, wi, wo):
    x = np.ascontiguousarray(
        np.asarray(hidden_states, dtype=np.float32).reshape(SEQ, D_MODEL)
    )
    wr = np.ascontiguousarray(np.asarray(w_router, dtype=np.float32))
    wi = np.asarray(wi, dtype=np.float32)
    wo = np.asarray(wo, dtype=np.float32)

    eidx, counts, c_pad = _dispatch(x, wr)

    key = (c_pad, DTYPE)
    nc = _CACHE.get(key)
    if nc is None:
        nc = _build(c_pad, DTYPE)
        _CACHE[key] = nc

    np_ff = {"fp16": np.float16, "bf16": None, "f32r": np.float32}[DTYPE]
    if DTYPE == "bf16":
        import ml_dtypes

        np_ff = ml_dtypes.bfloat16

    tok_lists = [np.nonzero(eidx == e)[0] for e in range(NUM_EXPERTS)]
    in_maps = []
    for e in range(NUM_EXPERTS):
        toks = tok_lists[e]
        xg = np.zeros((c_pad, D_MODEL), np.float32)
        xg[: len(toks)] = x[toks]
        in_maps.append(
            {
                "xg_t": np.ascontiguousarray(xg.T).astype(np_ff),
                "xs_t": np.ascontiguousarray(x[e * SHARD : (e + 1) * SHARD].T),
                "wr": wr,
                "wr_r": wr.astype(np_ff),
                "wi": np.ascontiguousarray(wi[e]).astype(np_ff),
                "wo": np.ascontiguousarray(wo[e]).astype(np_ff),
            }
        )

    from concourse.bass_utils import run_bass_kernel_spmd

    global LAST_RESULTS
    res = run_bass_kernel_spmd(
        nc,
        in_maps,
        list(range(N_CORES)),
        trace=PROFILE,
        trace_cores=TRACE_CORES,
    )
    LAST_RESULTS = res

    out = np.empty((SEQ, D_MODEL), np.float32)
    lg_full = np.empty((SEQ, NUM_EXPERTS), np.float32)
    ix_full = np.empty((SEQ,), np.int32)
    for e in range(NUM_EXPERTS):
        r = res.results[e]
        toks = tok_lists[e]
        out[toks] = r["y_out"].reshape(-1, D_MODEL)[: len(toks)]
        lg_full[e * SHARD : (e + 1) * SHARD] = r["lg_out"].reshape(SHARD, NUM_EXPERTS)
        ix_full[e * SHARD : (e + 1) * SHARD] = r["ix_out"].reshape(SHARD)

    return (
        out.reshape(BATCH, SEQ, D_MODEL),
        (
            lg_full.reshape(BATCH, SEQ, NUM_EXPERTS),
            ix_full.reshape(BATCH, SEQ),
        ),
    )
